# revision 5
# baseline (speedup 1.0000x reference)
import sys
import time

import numpy as np

sys.path.insert(0, "/opt/trn_rl_repo")

import concourse.bass as bass
import concourse.mybir as mybir
import concourse.tile as tile
from concourse import bacc
from concourse.bass_utils import run_bass_kernel_spmd
from concourse.masks import make_identity

B, T, D, V = 64, 512, 256, 32000
G4 = 4 * D  # 1024
NCORES = 8
BL = B // NCORES  # 8 sequences per core
CH = 2 * BL       # 16 chains per core (8 fwd + 8 rev)
ROWS = T * CH     # 8192 gather rows per core
NTILES = ROWS // 128  # 64 gather tiles
NBLK = ROWS // 512    # 16 GEMM blocks

F32 = mybir.dt.float32
BF16 = mybir.dt.bfloat16
I32 = mybir.dt.int32

_cache = {}


def _build_nc():
    nc = bacc.Bacc("TRN2", target_bir_lowering=False, debug=False)
    emb_d = nc.declare_dram_parameter("emb", [V, D], F32, isOutput=False)
    idx_d = nc.declare_dram_parameter("widx", [128, NTILES], I32, isOutput=False)
    wih_d = nc.declare_dram_parameter("wih", [128, 2048], F32, isOutput=False)
    whh_d = nc.declare_dram_parameter("whh", [128, 2048], F32, isOutput=False)
    bias_d = nc.declare_dram_parameter("bias", [128, 8], F32, isOutput=False)
    hout_d = nc.declare_dram_parameter("hout", [128, T * 32], BF16, isOutput=True)

    with tile.TileContext(nc) as tc:
        with (
            tc.tile_pool(name="const", bufs=1) as const,
            tc.tile_pool(name="wstage", bufs=2) as wstage,
            tc.tile_pool(name="xp", bufs=3) as xp,
            tc.tile_pool(name="xtp", bufs=2) as xtp,
            tc.tile_pool(name="tps", bufs=2, space="PSUM") as tps,
            tc.tile_pool(name="gemmps", bufs=2, space="PSUM") as gemmps,
            tc.tile_pool(name="scanps", bufs=2, space="PSUM") as scanps,
            tc.tile_pool(name="sc", bufs=3) as sc,
        ):
            # --- constants / weights ---
            ident = const.tile([128, 128], F32)
            make_identity(nc, ident[:])
            idx_sb = const.tile([128, NTILES], I32)
            nc.sync.dma_start(idx_sb[:], idx_d[:])
            bias_sb = const.tile([128, 8], F32)
            nc.sync.dma_start(bias_sb[:], bias_d[:])

            wih_sb = const.tile([128, 2048], BF16)
            whh_sb = const.tile([128, 2048], BF16)
            for half in range(2):
                s = wstage.tile([128, 1024], F32, tag="wst")
                nc.sync.dma_start(s[:], wih_d[:, half * 1024:(half + 1) * 1024])
                nc.vector.tensor_copy(out=wih_sb[:, half * 1024:(half + 1) * 1024], in_=s[:])
            for half in range(2):
                s = wstage.tile([128, 1024], F32, tag="wst")
                nc.sync.dma_start(s[:], whh_d[:, half * 1024:(half + 1) * 1024])
                nc.vector.tensor_copy(out=whh_sb[:, half * 1024:(half + 1) * 1024], in_=s[:])

            # Gx: input-side gate pre-activations, bf16, col layout per step = m*16 + j
            gx_sb = const.tile([128, T * 128], BF16)
            # h history: block t+1 = h state after step t; col = c*16 + j
            h_hist = const.tile([128, (T + 1) * 32], BF16)
            nc.gpsimd.memset(h_hist[:, 0:32], 0.0)

            # --- phase A: gather -> transpose -> GEMM -> Gx ---
            for blk in range(NBLK):
                xT0 = xtp.tile([128, 512], BF16, tag="xT0")
                xT1 = xtp.tile([128, 512], BF16, tag="xT1")
                xTs = (xT0, xT1)
                for s4 in range(4):
                    xt = xp.tile([128, D], F32, tag="xt")
                    nc.gpsimd.indirect_dma_start(
                        out=xt[:],
                        out_offset=None,
                        in_=emb_d[:],
                        in_offset=bass.IndirectOffsetOnAxis(
                            ap=idx_sb[:, blk * 4 + s4: blk * 4 + s4 + 1], axis=0
                        ),
                    )
                    for c in range(2):
                        pt = tps.tile([128, 128], F32, tag="pt")
                        nc.tensor.transpose(pt[:], xt[:, c * 128:(c + 1) * 128], ident[:])
                        nc.vector.tensor_copy(
                            out=xTs[c][:, s4 * 128:(s4 + 1) * 128], in_=pt[:]
                        )
                for m in range(8):
                    gp = gemmps.tile([128, 512], F32, tag="gp")
                    for c in range(2):
                        nc.tensor.matmul(
                            gp[:],
                            wih_sb[:, (m * 2 + c) * 128:(m * 2 + c + 1) * 128],
                            xTs[c][:],
                            start=(c == 0),
                            stop=(c == 1),
                        )
                    dst = (
                        gx_sb[:]
                        .rearrange("p (t m j) -> p t m j", m=8, j=16)
                        [:, blk * 32:(blk + 1) * 32, m, :]
                    )
                    src = gp[:].rearrange("p (t j) -> p t j", j=16)
                    nc.vector.tensor_scalar(
                        out=dst,
                        in0=src,
                        scalar1=bias_sb[:, m:m + 1],
                        scalar2=None,
                        op0=mybir.AluOpType.add,
                    )

            # --- phase B: 512-step fused fwd+rev LSTM scan ---
            cprev = sc.tile([128, 32], F32, tag="c")
            nc.gpsimd.memset(cprev[:], 0.0)
            for t in range(T):
                gpsum = scanps.tile([128, 128], F32, tag="gps")
                hprev = h_hist[:, t * 32:(t + 1) * 32]
                for m in range(8):
                    for c in range(2):
                        nc.tensor.matmul(
                            gpsum[:, m * 16:(m + 1) * 16],
                            whh_sb[:, (m * 2 + c) * 128:(m * 2 + c + 1) * 128],
                            hprev[:, c * 16:(c + 1) * 16],
                            start=(c == 0),
                            stop=(c == 1),
                        )
                gs = sc.tile([128, 128], F32, tag="gs")
                nc.vector.tensor_add(gs[:], gpsum[:], gx_sb[:, t * 128:(t + 1) * 128])
                sact = sc.tile([128, 96], F32, tag="sact")
                nc.scalar.activation(sact[:], gs[:, 0:96], mybir.ActivationFunctionType.Sigmoid)
                gact = sc.tile([128, 32], F32, tag="gact")
                nc.scalar.activation(gact[:], gs[:, 96:128], mybir.ActivationFunctionType.Tanh)
                t1 = sc.tile([128, 32], F32, tag="t1")
                nc.vector.tensor_mul(t1[:], sact[:, 32:64], cprev[:])
                t2 = sc.tile([128, 32], F32, tag="t2")
                nc.vector.tensor_mul(t2[:], sact[:, 0:32], gact[:])
                cnew = sc.tile([128, 32], F32, tag="c")
                nc.vector.tensor_add(cnew[:], t1[:], t2[:])
                tcc = sc.tile([128, 32], F32, tag="tcc")
                nc.scalar.activation(tcc[:], cnew[:], mybir.ActivationFunctionType.Tanh)
                nc.vector.tensor_mul(
                    h_hist[:, (t + 1) * 32:(t + 2) * 32], sact[:, 64:96], tcc[:]
                )
                cprev = cnew

            # --- phase C: bulk output DMA ---
            nc.sync.dma_start(hout_d[:], h_hist[:, 32:])

    nc.compile()
    return nc


def kernel(words, lengths, emb, W_ih, W_hh, b_ih, b_hh):
    words = np.asarray(words).astype(np.int64)
    lengths = np.asarray(lengths).astype(np.int64)
    emb = np.ascontiguousarray(np.asarray(emb, dtype=np.float32))
    W_ih = np.asarray(W_ih, dtype=np.float32)
    W_hh = np.asarray(W_hh, dtype=np.float32)
    bias = (np.asarray(b_ih, dtype=np.float32) + np.asarray(b_hh, dtype=np.float32))

    # reorder gates [i,f,g,o] -> [i,f,o,g]
    perm = np.concatenate([np.arange(0, 512), np.arange(768, 1024), np.arange(512, 768)])
    W_ih_r = W_ih[perm]
    W_hh_r = W_hh[perm]
    bias_r = bias[perm]

    def tile_w(Wr):
        # lhsT tiles: W^T [D, 4D]; tile (m, c) at col (m*2+c)*128
        WT = Wr.T  # [256, 1024]
        cols = []
        for m in range(8):
            for c in range(2):
                cols.append(WT[c * 128:(c + 1) * 128, m * 128:(m + 1) * 128])
        return np.ascontiguousarray(np.concatenate(cols, axis=1))  # [128, 2048]

    wih_t = tile_w(W_ih_r)
    whh_t = tile_w(W_hh_r)
    bias_t = np.ascontiguousarray(bias_r.reshape(8, 128).T)  # [128, 8]

    # reverse-read indices per sequence: words_rev[b, t] = words[b, max(len-1-t, 0)]
    tt = np.arange(T)
    idx_rev = np.clip(lengths[:, None] - 1 - tt[None, :], 0, None)  # [B, T]
    words_rev = np.take_along_axis(words, idx_rev, axis=1)

    in_maps = []
    for ci in range(NCORES):
        b0 = ci * BL
        # flat gather order r = t*16 + j ; j<8 fwd, j>=8 rev
        wf = np.empty((T, CH), dtype=np.int32)
        wf[:, 0:BL] = words[b0:b0 + BL].T
        wf[:, BL:CH] = words_rev[b0:b0 + BL].T
        wflat = wf.reshape(-1)  # [8192]
        widx = np.ascontiguousarray(wflat.reshape(NTILES, 128).T).astype(np.int32)
        in_maps.append({
            "emb": emb,
            "widx": widx,
            "wih": wih_t,
            "whh": whh_t,
            "bias": bias_t,
        })

    key = "nc"
    if key not in _cache:
        _cache[key] = _build_nc()
    nc = _cache[key]

    t0 = time.time()
    res = run_bass_kernel_spmd(nc, in_maps, list(range(NCORES)))
    t1 = time.time()
    print(f"[kernel] spmd run wall: {(t1 - t0)*1e3:.1f} ms", file=sys.stderr)

    # assemble
    b_h_all = np.zeros((T, B, 2 * D), dtype=np.float32)
    for ci in range(NCORES):
        b0 = ci * BL
        hout = np.asarray(res.results[ci]["hout"]).astype(np.float32)  # [128, T*32]
        # [128 p, t, c, j] -> [t, j, c, p];  d = c*128 + p
        harr = hout.reshape(128, T, 2, 16).transpose(1, 3, 2, 0).reshape(T, 16, 256)
        b_h_all[:, b0:b0 + BL, 0:D] = harr[:, 0:BL, :]
        b_h_all[:, b0:b0 + BL, D:2 * D] = harr[:, BL:CH, :]

    # zero masked steps (t >= len)
    mask = (tt[:, None] < lengths[None, :]).astype(np.float32)  # [T, B]
    b_h_all *= mask[:, :, None]

    # out = final reverse hidden of last batch element = rev output at step len-1
    lb = int(lengths[B - 1])
    out = b_h_all[lb - 1, B - 1, D:2 * D].reshape(1, D).copy()
    return out, b_h_all
